# revision 8
# baseline (speedup 1.0000x reference)
"""AttFlowLayer (BiDAF attention-flow) Trainium2 kernel, data-parallel over batch.

Problem: B=8, Lc=2048, Lq=1024, D=256 (all fp32).
  S[b,i,j] = c_i.w_c + q_j.w_q + sum_d c[b,i,d]*w_m[d]*q[j,d]
  P = softmax(S, axis=i);  A[b,j,i] = P[b,i,j] * qmask[j]
  H[b,j,:] = sum_i A[b,j,i] * c[b,i,:]
  G = [c, c * colsum(A)]   (colsum over j)

Key algebra used on-device (per batch element = per core):
  * the q_j.w_q term is constant over i, so it cancels in the softmax -> dropped.
  * c_i.w_c folds into the S matmul:  S'[i,j] = sum_d Ct[d,i] * (q[j,d]*w_m[d] + w_c[d])
  * softmax normalization is deferred: E = exp(S'), colsumE[j] = sum_i E[i,j]
    computed early by ones-vector matmuls, then everything is scaled by
    r[j] = qmask[j]/colsumE[j].

One batch element per NeuronCore; no collectives.
"""

import sys

if "/opt/trn_rl_repo" not in sys.path:
    sys.path.insert(0, "/opt/trn_rl_repo")

from contextlib import ExitStack

import numpy as np

import concourse.tile as tile
from concourse import bacc, mybir
from concourse.bass_utils import run_bass_kernel_spmd

LC, LQ, D, B = 2048, 1024, 256, 8
P = 128
NT_I, NT_J, ND = LC // P, LQ // P, D // P  # 16, 8, 2
F32, BF16 = mybir.dt.float32, mybir.dt.bfloat16
AL = mybir.AluOpType
AF = mybir.ActivationFunctionType
AX = mybir.AxisListType

_CACHED_NC = None

# experiment knobs (kernel defaults are the tuned configuration)
OPT = {"tpose": "dma", "cse": "interleave", "ps_s_bufs": 4}


def _program(tc, ctx_d, q_d, w_d, g_d, h_d):
    nc = tc.nc
    ctx_re = ctx_d.rearrange("(t p) d -> p t d", p=P)  # [128, 16, 256]
    q_re = q_d.rearrange("(t p) d -> p t d", p=P)  # [128, 8, 256]
    g_re = g_d.rearrange("(t p) e -> p t e", p=P)  # [128, 16, 512]
    h_re = h_d.rearrange("(t p) d -> p t d", p=P)  # [128, 8, 256]

    with ExitStack() as ex:
        const = ex.enter_context(tc.tile_pool(name="const", bufs=1))
        sb = ex.enter_context(tc.tile_pool(name="sb", bufs=1))
        work = ex.enter_context(tc.tile_pool(name="work", bufs=2))
        ps_s = ex.enter_context(
            tc.tile_pool(name="ps_s", bufs=int(OPT["ps_s_bufs"]), space="PSUM")
        )
        ps_h = ex.enter_context(tc.tile_pool(name="ps_h", bufs=2, space="PSUM"))
        ps_c = ex.enter_context(tc.tile_pool(name="ps_c", bufs=1, space="PSUM"))
        dram = ex.enter_context(tc.tile_pool(name="dram", bufs=1, space="DRAM"))

        ones_row = const.tile([1, P], F32, tag="ones_row", name="ones_row")
        nc.vector.memset(ones_row, 1.0)
        ones_col = const.tile([P, 1], BF16, tag="ones_col", name="ones_col")
        nc.gpsimd.memset(ones_col, 1.0)

        # w_c / w_m in column layout [128, ND] (partition = d mod 128)
        wc_col = const.tile([P, ND], F32, tag="wc", name="wc_col")
        wm_col = const.tile([P, ND], F32, tag="wm", name="wm_col")
        with nc.allow_non_contiguous_dma(reason="tiny 1KB const load"):
            nc.sync.dma_start(wc_col, w_d[0:D].rearrange("(t p) -> p t", p=P))
            nc.sync.dma_start(wm_col, w_d[2 * D : 3 * D].rearrange("(t p) -> p t", p=P))

        # ---- loads + bf16 conversions + transposes (DMA xbar) ----
        with nc.named_scope("load"):
            q_sb = sb.tile([P, NT_J, D], F32, tag="q_sb", name="q_sb")
            for ch in range(2):
                sl = slice(ch * 4, (ch + 1) * 4)
                nc.sync.dma_start(q_sb[:, sl, :], q_re[:, sl, :])
            c_sb = sb.tile([P, NT_I, D], F32, tag="c_sb", name="c_sb")
            for ch in range(4):
                sl = slice(ch * 4, (ch + 1) * 4)
                nc.sync.dma_start(c_sb[:, sl, :], ctx_re[:, sl, :])

        with nc.named_scope("tpose"):
            use_dma_t = OPT["tpose"] == "dma"
            if not use_dma_t:
                from concourse.masks import make_identity

                ident = const.tile([P, P], F32, tag="ident", name="ident")
                make_identity(nc, ident)
                ps_t = ex.enter_context(
                    tc.tile_pool(name="ps_t", bufs=2, space="PSUM")
                )
            # q in bf16, then transpose, then qpp = qt*w_m + w_c
            q_b16 = sb.tile([P, NT_J, D], BF16, tag="q_b16", name="q_b16")
            for m in range(NT_J):
                nc.scalar.copy(q_b16[:, m, :], q_sb[:, m, :])
            qt = [
                sb.tile([P, LQ], BF16, tag=f"qt{dt}", name=f"qt{dt}")
                for dt in range(ND)
            ]
            for m in range(NT_J):
                for dt in range(ND):
                    if use_dma_t:
                        nc.sync.dma_start(
                            qt[dt][:, m * P : (m + 1) * P],
                            q_b16[:, m, dt * P : (dt + 1) * P],
                            transpose=True,
                        )
                    else:
                        pt = ps_t.tile([P, P], F32, tag="pt", name="pt")
                        nc.tensor.transpose(
                            pt, q_sb[:, m, dt * P : (dt + 1) * P], ident
                        )
                        nc.vector.tensor_copy(qt[dt][:, m * P : (m + 1) * P], pt)
            qpp = [
                const.tile([P, LQ], BF16, tag=f"qpp{dt}", name=f"qpp{dt}")
                for dt in range(ND)
            ]
            for dt in range(ND):
                nc.vector.tensor_scalar(
                    qpp[dt],
                    qt[dt],
                    wm_col[:, dt : dt + 1],
                    wc_col[:, dt : dt + 1],
                    AL.mult,
                    AL.add,
                )

            # c in bf16 (also the rhs of the H matmul), transposed
            cpp = [
                sb.tile([P, D], BF16, tag=f"cpp{i}", name=f"cpp{i}")
                for i in range(NT_I)
            ]
            ct = [
                const.tile([P, LC], BF16, tag=f"ct{dt}", name=f"ct{dt}")
                for dt in range(ND)
            ]
            for i in range(NT_I):
                nc.scalar.copy(cpp[i], c_sb[:, i, :])
                for dt in range(ND):
                    if use_dma_t:
                        nc.sync.dma_start(
                            ct[dt][:, i * P : (i + 1) * P],
                            cpp[i][:, dt * P : (dt + 1) * P],
                            transpose=True,
                        )
                    else:
                        pt = ps_t.tile([P, P], F32, tag="pt", name="pt")
                        nc.tensor.transpose(
                            pt, c_sb[:, i, dt * P : (dt + 1) * P], ident
                        )
                        nc.vector.tensor_copy(ct[dt][:, i * P : (i + 1) * P], pt)

        # qmask[j] = (sum_d q[j,d]) != 0, in column then row form
        qmask = const.tile([P, NT_J], F32, tag="qmask", name="qmask")
        for m in range(NT_J):
            qs = work.tile([P, 1], F32, tag="qs", name="qs")
            nc.vector.tensor_reduce(qs, q_sb[:, m, :], AX.X, AL.add)
            nc.vector.tensor_scalar(qmask[:, m : m + 1], qs, 0.0, None, AL.not_equal)
        qm_d = dram.tile([P, NT_J], F32, tag="qm_d", name="qm_d")
        nc.sync.dma_start(qm_d, qmask)
        qmask_row = const.tile([1, LQ], F32, tag="qmask_row", name="qmask_row")
        with nc.allow_non_contiguous_dma(reason="tiny 4KB gather"):
            nc.sync.dma_start(
                qmask_row.rearrange("o (t p) -> o t p", p=P),
                qm_d.rearrange("p t -> t p")[None, :, :],
            )

        # ---- S matmul + exp, with colsumE ones-matmuls interleaved ----
        # E[i][:, j] = exp(sum_dt ct[dt][:,i-chunk].T @ qpp[dt][:, j-chunk])
        # cse[jc] += ones.T @ E[i][:, jc]   (accumulated over i)
        e_sb = [
            sb.tile([P, LQ], BF16, tag=f"e{i}", name=f"e{i}") for i in range(NT_I)
        ]
        cse_ps = [
            ps_c.tile([1, 512], F32, tag=f"cse{jc}", name=f"cse{jc}")
            for jc in range(2)
        ]

        def emit_cse(i):
            for jc in range(2):
                nc.tensor.matmul(
                    cse_ps[jc],
                    ones_col,
                    e_sb[i][:, jc * 512 : (jc + 1) * 512],
                    start=(i == 0),
                    stop=(i == NT_I - 1),
                )

        with nc.named_scope("smm"):
            for i in range(NT_I):
                pss = [
                    ps_s.tile([P, 512], F32, tag="ps_s", name="pss") for _ in range(2)
                ]
                for dt in range(ND):
                    for jc in range(2):
                        nc.tensor.matmul(
                            pss[jc],
                            ct[dt][:, i * P : (i + 1) * P],
                            qpp[dt][:, jc * 512 : (jc + 1) * 512],
                            start=(dt == 0),
                            stop=(dt == ND - 1),
                        )
                for jc in range(2):
                    nc.scalar.activation(
                        e_sb[i][:, jc * 512 : (jc + 1) * 512], pss[jc], AF.Exp
                    )
                if OPT["cse"] == "interleave" and i >= 2:
                    emit_cse(i - 2)
            if OPT["cse"] == "interleave":
                emit_cse(NT_I - 2)
                emit_cse(NT_I - 1)
            else:
                for i in range(NT_I):
                    emit_cse(i)

        # ---- r = qmask/colsumE in row + column + broadcast forms ----
        with nc.named_scope("rprep"):
            colse = work.tile([1, LQ], F32, tag="colse", name="colse", bufs=1)
            for jc in range(2):
                nc.vector.tensor_copy(colse[:, jc * 512 : (jc + 1) * 512], cse_ps[jc])
            rrec = work.tile([1, LQ], F32, tag="rrec", name="rrec", bufs=1)
            nc.vector.reciprocal(rrec, colse)
            r_row = const.tile([1, LQ], F32, tag="r_row", name="r_row")
            nc.vector.tensor_tensor(r_row, rrec, qmask_row, AL.mult)
            # broadcast r over partitions: rb = ones_row.T @ r_row  (K=1 matmul)
            rb = const.tile([P, LQ], BF16, tag="rb", name="rb")
            for jc in range(2):
                pr = ps_s.tile([P, 512], F32, tag="ps_s", name="pr")
                nc.tensor.matmul(
                    pr,
                    ones_row,
                    r_row[0:1, jc * 512 : (jc + 1) * 512],
                    start=True,
                    stop=True,
                )
                nc.vector.tensor_copy(rb[:, jc * 512 : (jc + 1) * 512], pr)
            # r in column form for the H normalization
            r_d = dram.tile([1, LQ], F32, tag="r_d", name="r_d")
            nc.sync.dma_start(r_d, r_row)
            r_col = const.tile([P, NT_J], F32, tag="r_col", name="r_col")
            with nc.allow_non_contiguous_dma(reason="tiny 4KB scatter"):
                nc.sync.dma_start(
                    r_col,
                    r_d.rearrange("o (t p) -> p (o t)", p=P),
                )

        # ---- H matmul + normalize (scale by r[j]) ----
        with nc.named_scope("hmm"):
            for m in range(NT_J):
                ph = ps_h.tile([P, D], F32, tag="ps_h", name="ph")
                for i in range(NT_I):
                    nc.tensor.matmul(
                        ph,
                        e_sb[i][:, m * P : (m + 1) * P],
                        cpp[i],
                        start=(i == 0),
                        stop=(i == NT_I - 1),
                    )
                hs = work.tile([P, D], F32, tag="hs", name="hs")
                nc.scalar.activation(
                    hs, ph, AF.Copy, bias=0.0, scale=r_col[:, m : m + 1]
                )
                nc.sync.dma_start(h_re[:, m, :], hs)

        # ---- colsum_P[i] = sum_j E[i,j]*r[j];  G[:, D:2D] = c * colsum_P ----
        with nc.named_scope("tail"):
            for i in range(NT_I):
                scr = work.tile([P, LQ], BF16, tag="scr", name="scr")
                nc.gpsimd.tensor_tensor(scr, e_sb[i], rb, AL.mult)
                colp = work.tile([P, 1], F32, tag="colp", name="colp")
                nc.vector.tensor_reduce(colp, scr, AX.X, AL.add)
                ga = work.tile([P, D], F32, tag="ga", name="ga")
                nc.scalar.activation(
                    ga, c_sb[:, i, :], AF.Copy, bias=0.0, scale=colp
                )
                nc.sync.dma_start(g_re[:, i, D : 2 * D], ga)

        # G[:, 0:D] = context (pass-through), emitted last = low priority
        with nc.named_scope("gleft"):
            for ch in range(4):
                sl = slice(ch * 4, (ch + 1) * 4)
                nc.sync.dma_start(g_re[:, sl, 0:D], c_sb[:, sl, :])


def _build():
    nc = bacc.Bacc("TRN2", target_bir_lowering=False, debug=False, num_devices=B)
    ctx_d = nc.dram_tensor("ctx", [LC, D], F32, kind="ExternalInput").ap()
    q_d = nc.dram_tensor("q", [LQ, D], F32, kind="ExternalInput").ap()
    w_d = nc.dram_tensor("w", [3 * D], F32, kind="ExternalInput").ap()
    g_d = nc.dram_tensor("g", [LC, 2 * D], F32, kind="ExternalOutput").ap()
    h_d = nc.dram_tensor("h", [LQ, D], F32, kind="ExternalOutput").ap()
    with tile.TileContext(nc) as tc:
        _program(tc, ctx_d, q_d, w_d, g_d, h_d)
    nc.compile()
    return nc


def _get_nc():
    global _CACHED_NC
    if _CACHED_NC is None:
        _CACHED_NC = _build()
    return _CACHED_NC


def _make_in_maps(context, query, w_alpha):
    context = np.asarray(context, dtype=np.float32)
    query = np.ascontiguousarray(np.asarray(query, dtype=np.float32))
    w_alpha = np.ascontiguousarray(np.asarray(w_alpha, dtype=np.float32))
    return [
        {"ctx": np.ascontiguousarray(context[b]), "q": query, "w": w_alpha}
        for b in range(B)
    ]


def _run_spmd(in_maps, **kw):
    return run_bass_kernel_spmd(_get_nc(), in_maps, core_ids=list(range(B)), **kw)


def kernel(context, query, w_alpha):
    res = _run_spmd(_make_in_maps(context, query, w_alpha))
    G = np.stack([res.results[b]["g"] for b in range(B)])
    H = np.stack([res.results[b]["h"] for b in range(B)])
    return (G, H)


# revision 9
# speedup vs baseline: 1.1968x; 1.1968x over previous
"""AttFlowLayer (BiDAF attention-flow) Trainium2 kernel, data-parallel over batch.

Problem: B=8, Lc=2048, Lq=1024, D=256 (all fp32).
  S[b,i,j] = c_i.w_c + q_j.w_q + sum_d c[b,i,d]*w_m[d]*q[j,d]
  P = softmax(S, axis=i);  A[b,j,i] = P[b,i,j] * qmask[j]
  H[b,j,:] = sum_i A[b,j,i] * c[b,i,:]
  G = [c, c * colsum(A)]   (colsum over j)

Key algebra used on-device (per batch element = per core):
  * the q_j.w_q term is constant over i, so it cancels in the softmax -> dropped.
  * c_i.w_c folds into the S matmul:  S'[i,j] = sum_d Ct[d,i] * (q[j,d]*w_m[d] + w_c[d])
  * softmax normalization is deferred: E = exp(S'), colsumE[j] = sum_i E[i,j]
    computed early by ones-vector matmuls, then everything is scaled by
    r[j] = qmask[j]/colsumE[j].

One batch element per NeuronCore; no collectives.
"""

import sys

if "/opt/trn_rl_repo" not in sys.path:
    sys.path.insert(0, "/opt/trn_rl_repo")

from contextlib import ExitStack

import numpy as np

import concourse.tile as tile
from concourse import bacc, mybir
from concourse.bass_utils import run_bass_kernel_spmd

LC, LQ, D, B = 2048, 1024, 256, 8
P = 128
NT_I, NT_J, ND = LC // P, LQ // P, D // P  # 16, 8, 2
F32, BF16 = mybir.dt.float32, mybir.dt.bfloat16
AL = mybir.AluOpType
AF = mybir.ActivationFunctionType
AX = mybir.AxisListType

_CACHED_NC = None

# experiment knobs (kernel defaults are the tuned configuration)
OPT = {"tpose": "dma", "cse": "interleave", "ps_s_bufs": 4}


def _program(tc, ctx_d, q_d, w_d, g_d, h_d):
    nc = tc.nc
    ctx_re = ctx_d.rearrange("(t p) d -> p t d", p=P)  # [128, 16, 256]
    q_re = q_d.rearrange("(t p) d -> p t d", p=P)  # [128, 8, 256]
    g_re = g_d.rearrange("(t p) e -> p t e", p=P)  # [128, 16, 512]
    h_re = h_d.rearrange("(t p) d -> p t d", p=P)  # [128, 8, 256]

    with ExitStack() as ex:
        const = ex.enter_context(tc.tile_pool(name="const", bufs=1))
        sb = ex.enter_context(tc.tile_pool(name="sb", bufs=1))
        work = ex.enter_context(tc.tile_pool(name="work", bufs=2))
        ps_s = ex.enter_context(
            tc.tile_pool(name="ps_s", bufs=int(OPT["ps_s_bufs"]), space="PSUM")
        )
        ps_h = ex.enter_context(tc.tile_pool(name="ps_h", bufs=2, space="PSUM"))
        ps_c = ex.enter_context(tc.tile_pool(name="ps_c", bufs=1, space="PSUM"))
        dram = ex.enter_context(tc.tile_pool(name="dram", bufs=1, space="DRAM"))

        ones_row = const.tile([1, P], F32, tag="ones_row", name="ones_row")
        nc.vector.memset(ones_row, 1.0)
        ones_col = const.tile([P, 1], BF16, tag="ones_col", name="ones_col")
        nc.gpsimd.memset(ones_col, 1.0)

        # w_c / w_m in column layout [128, ND] (partition = d mod 128)
        wc_col = const.tile([P, ND], F32, tag="wc", name="wc_col")
        wm_col = const.tile([P, ND], F32, tag="wm", name="wm_col")
        with nc.allow_non_contiguous_dma(reason="tiny 1KB const load"):
            nc.sync.dma_start(wc_col, w_d[0:D].rearrange("(t p) -> p t", p=P))
            nc.sync.dma_start(wm_col, w_d[2 * D : 3 * D].rearrange("(t p) -> p t", p=P))

        # ---- loads + bf16 conversions + transposes (DMA xbar) ----
        with nc.named_scope("load"):
            q_sb = sb.tile([P, NT_J, D], F32, tag="q_sb", name="q_sb")
            for ch in range(2):
                sl = slice(ch * 4, (ch + 1) * 4)
                nc.sync.dma_start(q_sb[:, sl, :], q_re[:, sl, :])
            c_sb = sb.tile([P, NT_I, D], F32, tag="c_sb", name="c_sb")
            for ch in range(4):
                sl = slice(ch * 4, (ch + 1) * 4)
                nc.sync.dma_start(c_sb[:, sl, :], ctx_re[:, sl, :])

        with nc.named_scope("tpose"):
            # bf16 copies bounce through DRAM, then the hardware xbar
            # transposes each [*, 128]-column block DRAM->SBUF in one DMA.
            q_b16 = sb.tile([P, NT_J, D], BF16, tag="q_b16", name="q_b16")
            qb_dram = dram.tile([LQ, D], BF16, tag="qb_dram", name="qb_dram")
            qb_re = qb_dram.rearrange("(t p) d -> p t d", p=P)
            for m in range(NT_J):
                nc.scalar.copy(q_b16[:, m, :], q_sb[:, m, :])
                nc.sync.dma_start(qb_re[:, m, :], q_b16[:, m, :])
            qt = [
                sb.tile([P, LQ], BF16, tag=f"qt{dt}", name=f"qt{dt}")
                for dt in range(ND)
            ]
            for dt in range(ND):
                nc.sync.dma_start(
                    qt[dt], qb_dram[:, dt * P : (dt + 1) * P], transpose=True
                )
            qpp = [
                const.tile([P, LQ], BF16, tag=f"qpp{dt}", name=f"qpp{dt}")
                for dt in range(ND)
            ]
            for dt in range(ND):
                nc.vector.tensor_scalar(
                    qpp[dt],
                    qt[dt],
                    wm_col[:, dt : dt + 1],
                    wc_col[:, dt : dt + 1],
                    AL.mult,
                    AL.add,
                )

            # c in bf16 (also the rhs of the H matmul), transposed the same way
            cpp = [
                sb.tile([P, D], BF16, tag=f"cpp{i}", name=f"cpp{i}")
                for i in range(NT_I)
            ]
            cb_dram = dram.tile([LC, D], BF16, tag="cb_dram", name="cb_dram")
            cb_re = cb_dram.rearrange("(t p) d -> p t d", p=P)
            for i in range(NT_I):
                nc.scalar.copy(cpp[i], c_sb[:, i, :])
                nc.sync.dma_start(cb_re[:, i, :], cpp[i])
            ct = [
                const.tile([P, LC], BF16, tag=f"ct{dt}", name=f"ct{dt}")
                for dt in range(ND)
            ]
            for dt in range(ND):
                nc.sync.dma_start(
                    ct[dt], cb_dram[:, dt * P : (dt + 1) * P], transpose=True
                )

        # qmask[j] = (sum_d q[j,d]) != 0, in column then row form
        qmask = const.tile([P, NT_J], F32, tag="qmask", name="qmask")
        for m in range(NT_J):
            qs = work.tile([P, 1], F32, tag="qs", name="qs")
            nc.vector.tensor_reduce(qs, q_sb[:, m, :], AX.X, AL.add)
            nc.vector.tensor_scalar(qmask[:, m : m + 1], qs, 0.0, None, AL.not_equal)
        qm_d = dram.tile([P, NT_J], F32, tag="qm_d", name="qm_d")
        nc.sync.dma_start(qm_d, qmask)
        qmask_row = const.tile([1, LQ], F32, tag="qmask_row", name="qmask_row")
        with nc.allow_non_contiguous_dma(reason="tiny 4KB gather"):
            nc.sync.dma_start(
                qmask_row.rearrange("o (t p) -> o t p", p=P),
                qm_d.rearrange("p t -> t p")[None, :, :],
            )

        # ---- S matmul + exp, with colsumE ones-matmuls interleaved ----
        # E[i][:, j] = exp(sum_dt ct[dt][:,i-chunk].T @ qpp[dt][:, j-chunk])
        # cse[jc] += ones.T @ E[i][:, jc]   (accumulated over i)
        e_sb = [
            sb.tile([P, LQ], BF16, tag=f"e{i}", name=f"e{i}") for i in range(NT_I)
        ]
        cse_ps = [
            ps_c.tile([1, 512], F32, tag=f"cse{jc}", name=f"cse{jc}")
            for jc in range(2)
        ]

        def emit_cse(i):
            for jc in range(2):
                nc.tensor.matmul(
                    cse_ps[jc],
                    ones_col,
                    e_sb[i][:, jc * 512 : (jc + 1) * 512],
                    start=(i == 0),
                    stop=(i == NT_I - 1),
                )

        with nc.named_scope("smm"):
            for i in range(NT_I):
                pss = [
                    ps_s.tile([P, 512], F32, tag="ps_s", name="pss") for _ in range(2)
                ]
                for dt in range(ND):
                    for jc in range(2):
                        nc.tensor.matmul(
                            pss[jc],
                            ct[dt][:, i * P : (i + 1) * P],
                            qpp[dt][:, jc * 512 : (jc + 1) * 512],
                            start=(dt == 0),
                            stop=(dt == ND - 1),
                        )
                for jc in range(2):
                    nc.scalar.activation(
                        e_sb[i][:, jc * 512 : (jc + 1) * 512], pss[jc], AF.Exp
                    )
                if OPT["cse"] == "interleave" and i >= 2:
                    emit_cse(i - 2)
            if OPT["cse"] == "interleave":
                emit_cse(NT_I - 2)
                emit_cse(NT_I - 1)
            else:
                for i in range(NT_I):
                    emit_cse(i)

        # ---- r = qmask/colsumE in row + column + broadcast forms ----
        with nc.named_scope("rprep"):
            colse = work.tile([1, LQ], F32, tag="colse", name="colse", bufs=1)
            for jc in range(2):
                nc.vector.tensor_copy(colse[:, jc * 512 : (jc + 1) * 512], cse_ps[jc])
            rrec = work.tile([1, LQ], F32, tag="rrec", name="rrec", bufs=1)
            nc.vector.reciprocal(rrec, colse)
            r_row = const.tile([1, LQ], F32, tag="r_row", name="r_row")
            nc.vector.tensor_tensor(r_row, rrec, qmask_row, AL.mult)
            # broadcast r over partitions: rb = ones_row.T @ r_row  (K=1 matmul)
            rb = const.tile([P, LQ], BF16, tag="rb", name="rb")
            for jc in range(2):
                pr = ps_s.tile([P, 512], F32, tag="ps_s", name="pr")
                nc.tensor.matmul(
                    pr,
                    ones_row,
                    r_row[0:1, jc * 512 : (jc + 1) * 512],
                    start=True,
                    stop=True,
                )
                nc.vector.tensor_copy(rb[:, jc * 512 : (jc + 1) * 512], pr)
            # r in column form for the H normalization
            r_d = dram.tile([1, LQ], F32, tag="r_d", name="r_d")
            nc.sync.dma_start(r_d, r_row)
            r_col = const.tile([P, NT_J], F32, tag="r_col", name="r_col")
            with nc.allow_non_contiguous_dma(reason="tiny 4KB scatter"):
                nc.sync.dma_start(
                    r_col,
                    r_d.rearrange("o (t p) -> p (o t)", p=P),
                )

        # ---- H matmul + normalize (scale by r[j]) ----
        with nc.named_scope("hmm"):
            for m in range(NT_J):
                ph = ps_h.tile([P, D], F32, tag="ps_h", name="ph")
                for i in range(NT_I):
                    nc.tensor.matmul(
                        ph,
                        e_sb[i][:, m * P : (m + 1) * P],
                        cpp[i],
                        start=(i == 0),
                        stop=(i == NT_I - 1),
                    )
                hs = work.tile([P, D], F32, tag="hs", name="hs")
                nc.scalar.activation(
                    hs, ph, AF.Copy, bias=0.0, scale=r_col[:, m : m + 1]
                )
                nc.sync.dma_start(h_re[:, m, :], hs)

        # ---- colsum_P[i] = sum_j E[i,j]*r[j];  G[:, D:2D] = c * colsum_P ----
        with nc.named_scope("tail"):
            for i in range(NT_I):
                scr = work.tile([P, LQ], BF16, tag="scr", name="scr")
                nc.gpsimd.tensor_tensor(scr, e_sb[i], rb, AL.mult)
                colp = work.tile([P, 1], F32, tag="colp", name="colp")
                nc.vector.tensor_reduce(colp, scr, AX.X, AL.add)
                ga = work.tile([P, D], F32, tag="ga", name="ga")
                nc.scalar.activation(
                    ga, c_sb[:, i, :], AF.Copy, bias=0.0, scale=colp
                )
                nc.sync.dma_start(g_re[:, i, D : 2 * D], ga)

        # G[:, 0:D] = context (pass-through), emitted last = low priority
        with nc.named_scope("gleft"):
            for ch in range(4):
                sl = slice(ch * 4, (ch + 1) * 4)
                nc.sync.dma_start(g_re[:, sl, 0:D], c_sb[:, sl, :])


def _build():
    nc = bacc.Bacc("TRN2", target_bir_lowering=False, debug=False, num_devices=B)
    ctx_d = nc.dram_tensor("ctx", [LC, D], F32, kind="ExternalInput").ap()
    q_d = nc.dram_tensor("q", [LQ, D], F32, kind="ExternalInput").ap()
    w_d = nc.dram_tensor("w", [3 * D], F32, kind="ExternalInput").ap()
    g_d = nc.dram_tensor("g", [LC, 2 * D], F32, kind="ExternalOutput").ap()
    h_d = nc.dram_tensor("h", [LQ, D], F32, kind="ExternalOutput").ap()
    with tile.TileContext(nc) as tc:
        _program(tc, ctx_d, q_d, w_d, g_d, h_d)
    nc.compile()
    return nc


def _get_nc():
    global _CACHED_NC
    if _CACHED_NC is None:
        _CACHED_NC = _build()
    return _CACHED_NC


def _make_in_maps(context, query, w_alpha):
    context = np.asarray(context, dtype=np.float32)
    query = np.ascontiguousarray(np.asarray(query, dtype=np.float32))
    w_alpha = np.ascontiguousarray(np.asarray(w_alpha, dtype=np.float32))
    return [
        {"ctx": np.ascontiguousarray(context[b]), "q": query, "w": w_alpha}
        for b in range(B)
    ]


def _run_spmd(in_maps, **kw):
    return run_bass_kernel_spmd(_get_nc(), in_maps, core_ids=list(range(B)), **kw)


def kernel(context, query, w_alpha):
    res = _run_spmd(_make_in_maps(context, query, w_alpha))
    G = np.stack([res.results[b]["g"] for b in range(B)])
    H = np.stack([res.results[b]["h"] for b in range(B)])
    return (G, H)
